# revision 1
# baseline (speedup 1.0000x reference)
"""Multi-head causal attention (B=2, T=2048, D=1024, H=16) on 8 Trainium2
NeuronCores.

Sharding: core c handles batch b = c//4 and head group g = c%4 (4 heads,
o-columns [256g, 256g+256)).  Host pre-transposes x and the weight slices so
every matmul operand arrives in contraction-major layout; each core computes
its partial output projection y_part = att_part @ W_o.T[cols] and the host
sums the 4 partials per batch and adds b_o.

Per-core device program (all matmuls in float32r):
  Q^T,K^T  [o,t] = wT.T @ x^T    (Q prescaled by 1/sqrt(64), biases folded)
  V        [t,o] = x^T.T @ wvT   (+ ones-column per head for softmax denom)
  S^T      [k,q] = K^T_h.T @ Q^T_h   (two heads packed in PE row groups)
  P = exp(S^T)   (causal: lower-tri blocks only, diag blocks masked)
  O^T[dv,q], D[q] = [V_h|1].T @ P    (denominator via the ones column)
  att^T = O^T * bcast(1/D)           (rank-1 PE broadcast matmul)
  y_part[t,:] = att^T.T @ woT
"""
import sys

for _p in ("/opt/trn_rl_repo", "/root/.axon_site/_ro/trn_rl_repo"):
    if _p not in sys.path:
        sys.path.insert(0, _p)

import numpy as np

import concourse.bass as bass
import concourse.tile as tile
from concourse import bacc, mybir

F32 = mybir.dt.float32
F32R = mybir.dt.float32r

N_CORES = 8
EMBED = 1024
NH_CORE = 4          # heads per core
DH = 64              # head dim
OC = NH_CORE * DH    # 256 o-columns per core
KC = EMBED // 128    # 8 contraction chunks
NO = OC // 128       # 2 o-tiles of 128


def build_body(tc, aps, T, skip=()):
    nc = tc.nc
    P = 128
    SPAN = min(512, T)
    NSPAN = T // SPAN
    TPS = SPAN // P      # k/q tiles per span
    NT = T // P

    xT, wqT, wkT, wvT, woT, bq, bk, bv, y = (
        aps["xT"], aps["wqT"], aps["wkT"], aps["wvT"], aps["woT"],
        aps["bq"], aps["bk"], aps["bv"], aps["y"],
    )

    sb = aps["sb_pool"]
    ps = aps["ps_pool"]

    Ident = mybir.ActivationFunctionType.Identity
    Exp = mybir.ActivationFunctionType.Exp

    # ---- constants ----
    ones_f = sb.tile([128, 128], F32, tag="onesf")
    nc.gpsimd.memset(ones_f[:], 1.0)
    ones_sb = sb.tile([128, 128], F32R, tag="ones")
    nc.vector.tensor_copy(ones_sb[:], ones_f[:])
    tri_sb = sb.tile([128, 128], F32, tag="tri")
    nc.gpsimd.memset(tri_sb[:], 1.0)
    # keep (q - k) >= 0 (k on partitions, q on free), else 0.0
    nc.gpsimd.affine_select(
        out=tri_sb[:], in_=tri_sb[:], compare_op=mybir.AluOpType.is_ge,
        fill=0.0, base=0, pattern=[[1, 128]], channel_multiplier=-1,
    )

    # ---- input loads ----
    wv_sb = sb.tile([128, KC, OC], F32R, tag="wv")
    nc.sync.dma_start(wv_sb[:], wvT.rearrange("(kc p) o -> p kc o", p=P))
    xc = []
    for kc in range(KC):
        t = sb.tile([128, T], F32R, tag="xt", bufs=KC)
        nc.sync.dma_start(t[:], xT.rearrange("(kc p) t -> kc p t", p=P)[kc])
        xc.append(t)
    wq_sb = sb.tile([128, KC, OC], F32R, tag="wq")
    nc.sync.dma_start(wq_sb[:], wqT.rearrange("(kc p) o -> p kc o", p=P))
    wk_sb = sb.tile([128, KC, OC], F32R, tag="wk")
    nc.sync.dma_start(wk_sb[:], wkT.rearrange("(kc p) o -> p kc o", p=P))
    wo_sb = sb.tile([128, NO, EMBED], F32R, tag="wo")
    nc.sync.dma_start(wo_sb[:], woT.rearrange("(kc p) o -> p kc o", p=P))
    bq_sb = sb.tile([128, NO], F32, tag="bq")
    nc.sync.dma_start(bq_sb[:], bq.rearrange("(mo p) -> p mo", p=P))
    bk_sb = sb.tile([128, NO], F32, tag="bk")
    nc.sync.dma_start(bk_sb[:], bk.rearrange("(mo p) -> p mo", p=P))
    bv_sb = sb.tile([1, OC], F32R, tag="bv")
    nc.sync.dma_start(bv_sb[:], bv.rearrange("(a o) -> a o", a=1))

    QT_sb = sb.tile([128, NO, T], F32R, tag="qt")
    KT_sb = sb.tile([128, NO, T], F32R, tag="kt")
    V_sb = sb.tile([128, NT, NH_CORE * (DH + 1)], F32R, tag="v")
    attT_sb = sb.tile([128, NO, T], F32R, tag="att")

    PS = 512  # per-head column stride inside PSUM tiles (bank isolation)
    add, mult = mybir.AluOpType.add, mybir.AluOpType.mult

    for s in range(NSPAN):
        # ---- V projection for this span's t-tiles (ones column per head) ----
        for ti in range(TPS * s, TPS * (s + 1)):
            pv = ps.tile([128, OC], F32, tag="B", bufs=4)
            for kc in range(KC):
                nc.tensor.matmul(
                    pv[:], xc[kc][:, ti * 128:(ti + 1) * 128], wv_sb[:, kc, :],
                    start=(kc == 0), stop=False,
                )
            nc.tensor.matmul(
                pv[:], ones_sb[0:1, 0:128], bv_sb[0:1, :],
                start=False, stop=True,
            )
            nc.vector.tensor_copy(
                V_sb[:, ti, :].rearrange("p (h d) -> p h d", d=DH + 1)[:, :, 0:DH],
                pv[:].rearrange("p (h d) -> p h d", d=DH),
            )
            nc.vector.tensor_copy(
                V_sb[:, ti, :].rearrange("p (h d) -> p h d", d=DH + 1)[:, :, DH:DH + 1],
                ones_sb[:, 0:NH_CORE].rearrange("p (h d) -> p h d", d=1),
            )

        # ---- Q/K projections for this span: out [o, t] ----
        for dst, wsb, bias_sb, is_q in (
            (QT_sb, wq_sb, bq_sb, True),
            (KT_sb, wk_sb, bk_sb, False),
        ):
            for mo in range(NO):
                pt = ps.tile([128, SPAN], F32, tag="B", bufs=4)
                for kc in range(KC):
                    nc.tensor.matmul(
                        pt[:],
                        wsb[:, kc, mo * 128:(mo + 1) * 128],
                        xc[kc][:, s * SPAN:(s + 1) * SPAN],
                        start=(kc == 0), stop=(kc == KC - 1),
                    )
                if is_q:
                    nc.vector.tensor_scalar(
                        dst[:, mo, s * SPAN:(s + 1) * SPAN], pt[:],
                        bias_sb[:, mo:mo + 1], 0.125, add, mult,
                    )
                else:
                    nc.vector.tensor_scalar_add(
                        dst[:, mo, s * SPAN:(s + 1) * SPAN], pt[:],
                        bias_sb[:, mo:mo + 1],
                    )

        # ---- attention for this span: head pairs share the PE array ----
        for hp in range(NH_CORE // 2 if "att" not in skip else 0):
            poh = [ps.tile([DH + 1, 512], F32, tag="B", bufs=4, name=f"po_{s}_{hp}_{i}") for i in range(2)]
            kts = list(range(TPS * s, TPS * s + TPS)) + list(range(0, TPS * s))
            for idx, kt in enumerate(kts):
                j = kt - TPS * s if kt >= TPS * s else None
                lo = 128 * j if j is not None else 0
                pstile = ps.tile([128, 2 * PS], F32, tag="A", bufs=2)
                for hh in range(2 if "smm" not in skip else 0):
                    h = 2 * hp + hh
                    bp = 64 * (h % 2)
                    nc.tensor.matmul(
                        pstile[:, PS * hh + lo:PS * hh + SPAN],
                        KT_sb[bp:bp + DH, h // 2, kt * 128:(kt + 1) * 128],
                        QT_sb[bp:bp + DH, h // 2, SPAN * s + lo:SPAN * (s + 1)],
                        start=True, stop=True,
                    )
                pb = sb.tile([128, 2 * SPAN], F32R, tag="p", bufs=4)
                if "exp" not in skip:
                    if lo == 0 and SPAN == PS:
                        nc.scalar.activation(pb[:], pstile[:], Exp)
                    else:
                        for hh in range(2):
                            nc.scalar.activation(
                                pb[:, SPAN * hh + lo:SPAN * hh + SPAN],
                                pstile[:, PS * hh + lo:PS * hh + SPAN],
                                Exp,
                            )
                if j is not None and "mask" not in skip:
                    for hh in range(2):
                        blk = pb[:, SPAN * hh + lo:SPAN * hh + lo + 128]
                        nc.vector.tensor_mul(blk, blk, tri_sb[:])
                for hh in range(2 if "pv" not in skip else 0):
                    h = 2 * hp + hh
                    nc.tensor.matmul(
                        poh[hh][0:DH + 1, lo:SPAN],
                        V_sb[:, kt, (DH + 1) * h:(DH + 1) * (h + 1)],
                        pb[:, SPAN * hh + lo:SPAN * hh + SPAN],
                        start=(idx == 0), stop=(idx == len(kts) - 1),
                    )
            # normalize: att^T = O^T * bcast(1 / D)
            rb = sb.tile([128, 2 * SPAN], F32R, tag="rb", bufs=2)
            bc = sb.tile([DH, 2 * SPAN], F32, tag="bc", bufs=2)
            for hh in range(2):
                with nc.allow_low_precision(reason="f32r recip for PE bcast"):
                    nc.vector.reciprocal(
                        rb[DH:DH + 1, SPAN * hh:SPAN * (hh + 1)],
                        poh[hh][DH:DH + 1, 0:SPAN],
                    )
                pbk = ps.tile([DH, 512], F32, tag="B", bufs=4)
                nc.tensor.matmul(
                    pbk[:, 0:SPAN],
                    ones_sb[64:65, 0:DH],
                    rb[64:65, SPAN * hh:SPAN * (hh + 1)],
                    start=True, stop=True,
                )
                nc.vector.tensor_copy(
                    bc[:, SPAN * hh:SPAN * (hh + 1)], pbk[:, 0:SPAN],
                )
                h = 2 * hp + hh
                bp = 64 * (h % 2)
                nc.vector.tensor_mul(
                    attT_sb[bp:bp + DH, h // 2, SPAN * s:SPAN * (s + 1)],
                    poh[hh][0:DH, 0:SPAN],
                    bc[:, SPAN * hh:SPAN * (hh + 1)],
                )

        # ---- output projection for this span: y[t, :] = att^T.T @ woT ----
        for ti in range(TPS * s, TPS * (s + 1) if "yproj" not in skip else TPS * s):
            for no in range(EMBED // 512):
                py = ps.tile([128, 512], F32, tag="B", bufs=4)
                for kc2 in range(NO):
                    nc.tensor.matmul(
                        py[:],
                        attT_sb[:, kc2, ti * 128:(ti + 1) * 128],
                        wo_sb[:, kc2, no * 512:(no + 1) * 512],
                        start=(kc2 == 0), stop=(kc2 == NO - 1),
                    )
                ysb = sb.tile([128, 512], F32, tag="y", bufs=3)
                if ti % 2 == 0:
                    nc.vector.tensor_copy(ysb[:], py[:])
                else:
                    nc.scalar.copy(ysb[:], py[:])
                nc.sync.dma_start(
                    y[ti * 128:(ti + 1) * 128, no * 512:(no + 1) * 512], ysb[:],
                )


def build_nc(T=2048, reps=1, skip=()):
    nc = bacc.Bacc("TRN2", target_bir_lowering=False, debug=False,
                   enable_asserts=False, num_devices=N_CORES)
    aps = {
        "xT": nc.dram_tensor("xT", (EMBED, T), F32R, kind="ExternalInput").ap(),
        "wqT": nc.dram_tensor("wqT", (EMBED, OC), F32R, kind="ExternalInput").ap(),
        "wkT": nc.dram_tensor("wkT", (EMBED, OC), F32R, kind="ExternalInput").ap(),
        "wvT": nc.dram_tensor("wvT", (EMBED, OC), F32R, kind="ExternalInput").ap(),
        "woT": nc.dram_tensor("woT", (OC, EMBED), F32R, kind="ExternalInput").ap(),
        "bq": nc.dram_tensor("bq", (OC,), F32, kind="ExternalInput").ap(),
        "bk": nc.dram_tensor("bk", (OC,), F32, kind="ExternalInput").ap(),
        "bv": nc.dram_tensor("bv", (OC,), F32R, kind="ExternalInput").ap(),
        "y": nc.dram_tensor("y", (T, EMBED), F32, kind="ExternalOutput").ap(),
    }
    with tile.TileContext(nc) as tc:
        with tc.tile_pool(name="sb", bufs=1) as sb, \
             tc.tile_pool(name="ps", bufs=1, space="PSUM") as ps:
            aps["sb_pool"] = sb
            aps["ps_pool"] = ps
            if reps == 1:
                build_body(tc, aps, T, skip=skip)
            else:
                hints = (mybir.EngineType.PE, mybir.EngineType.Activation,
                         mybir.EngineType.DVE, mybir.EngineType.SP,
                         mybir.EngineType.Pool)
                with tc.For_i(0, reps, 1, hint_engines=hints):
                    build_body(tc, aps, T, skip=skip)
    nc.compile()
    return nc


def shard_inputs(x, W_q, b_q, W_k, b_k, W_v, b_v, W_o, b_o=None):
    """Full inputs -> list of 8 per-core input dicts (all float32, C-order)."""
    in_maps = []
    for c in range(N_CORES):
        b, g = divmod(c, 4)
        sl = slice(OC * g, OC * (g + 1))
        in_maps.append({
            "xT": np.ascontiguousarray(x[b].T, dtype=np.float32),
            "wqT": np.ascontiguousarray(W_q[sl, :].T, dtype=np.float32),
            "wkT": np.ascontiguousarray(W_k[sl, :].T, dtype=np.float32),
            "wvT": np.ascontiguousarray(W_v[sl, :].T, dtype=np.float32),
            "woT": np.ascontiguousarray(W_o[:, sl].T, dtype=np.float32),
            "bq": np.ascontiguousarray(b_q[sl], dtype=np.float32),
            "bk": np.ascontiguousarray(b_k[sl], dtype=np.float32),
            "bv": np.ascontiguousarray(b_v[sl], dtype=np.float32),
        })
    return in_maps


def _make_runner(nc, n_cores=N_CORES):
    """Compile-once, run-many SPMD runner (mirrors bass2jax.run_bass_via_pjrt)."""
    import jax
    from jax.sharding import Mesh, PartitionSpec
    from jax.experimental.shard_map import shard_map
    from concourse import bass2jax

    bass2jax.install_neuronx_cc_hook()
    partition_name = nc.partition_id_tensor.name if nc.partition_id_tensor else None
    in_names, out_names, out_avals, zero_outs = [], [], [], []
    for alloc in nc.m.functions[0].allocations:
        if not isinstance(alloc, mybir.MemoryLocationSet):
            continue
        name = alloc.memorylocations[0].name
        if alloc.kind == "ExternalInput":
            if name != partition_name:
                in_names.append(name)
        elif alloc.kind == "ExternalOutput":
            out_names.append(name)
            shape = tuple(alloc.tensor_shape)
            dtype = mybir.dt.np(alloc.dtype)
            out_avals.append(jax.core.ShapedArray(shape, dtype))
            zero_outs.append(np.zeros(shape, dtype))
    n_params = len(in_names)
    n_outs = len(out_avals)
    in_names_all = list(in_names) + list(out_names)
    if partition_name is not None:
        in_names_all.append(partition_name)
    donate = tuple(range(n_params, n_params + n_outs))

    def _body(*args):
        operands = list(args)
        if partition_name is not None:
            operands.append(bass2jax.partition_id_tensor())
        outs = bass2jax._bass_exec_p.bind(
            *operands,
            out_avals=tuple(out_avals),
            in_names=tuple(in_names_all),
            out_names=tuple(out_names),
            lowering_input_output_aliases=(),
            sim_require_finite=True,
            sim_require_nnan=True,
            nc=nc,
        )
        return tuple(outs)

    devices = jax.devices()[:n_cores]
    assert len(devices) == n_cores
    mesh = Mesh(np.asarray(devices), ("core",))
    in_specs = (PartitionSpec("core"),) * (n_params + n_outs)
    out_specs = (PartitionSpec("core"),) * len(out_names)
    jitted = jax.jit(
        shard_map(_body, mesh=mesh, in_specs=in_specs, out_specs=out_specs,
                  check_rep=False),
        donate_argnums=donate, keep_unused=True,
    )

    from jax.sharding import NamedSharding

    class Runner:
        def __init__(self):
            self._in_dev = None
            self._out_dev = None

        def prepare(self, in_maps):
            per_core = [[np.asarray(m[name]) for name in in_names]
                        for m in in_maps]
            concat_in = [
                np.concatenate([per_core[c][i] for c in range(n_cores)], axis=0)
                for i in range(n_params)
            ]
            sh = NamedSharding(mesh, PartitionSpec("core"))
            self._in_dev = [jax.device_put(a, sh) for a in concat_in]
            concat_zeros = [np.concatenate([z] * n_cores, axis=0)
                            for z in zero_outs]
            self._out_dev = [jax.device_put(a, sh) for a in concat_zeros]
            for a in self._in_dev + self._out_dev:
                a.block_until_ready()

        def execute(self):
            outs = jitted(*self._in_dev, *self._out_dev)
            for a in outs:
                a.block_until_ready()
            self._out_dev = list(outs)

        def fetch(self):
            out_arrs = [np.asarray(a) for a in self._out_dev]
            results = []
            for c in range(n_cores):
                m = {}
                for i, name in enumerate(out_names):
                    per_len = out_arrs[i].shape[0] // n_cores
                    m[name] = out_arrs[i][c * per_len:(c + 1) * per_len]
                results.append(m)
            return results

        def run(self, in_maps):
            self.prepare(in_maps)
            self.execute()
            return self.fetch()

    return Runner()


_CACHE = {}


def _get_runner(T=2048, reps=1):
    key = (T, reps)
    if key not in _CACHE:
        nc = build_nc(T=T, reps=reps)
        _CACHE[key] = _make_runner(nc)
    return _CACHE[key]


def kernel(**inputs):
    inputs = {k: np.asarray(v, dtype=np.float32) for k, v in inputs.items()}
    x = inputs["x"]
    B, T, C = x.shape
    in_maps = shard_inputs(**inputs)
    runner = _get_runner(T=T, reps=1)
    results = runner.run(in_maps)
    out = np.zeros((B, T, C), dtype=np.float32)
    for c in range(N_CORES):
        out[c // 4] += results[c]["y"]
    out += inputs["b_o"]
    return out


if __name__ == "__main__":
    rng = np.random.default_rng(0)
    s = 1.0 / np.sqrt(EMBED)
    ins = {
        "x": rng.standard_normal((2, 2048, EMBED), dtype=np.float32),
        "W_q": rng.uniform(-s, s, (EMBED, EMBED)).astype(np.float32),
        "b_q": rng.uniform(-s, s, (EMBED,)).astype(np.float32),
        "W_k": rng.uniform(-s, s, (EMBED, EMBED)).astype(np.float32),
        "b_k": rng.uniform(-s, s, (EMBED,)).astype(np.float32),
        "W_v": rng.uniform(-s, s, (EMBED, EMBED)).astype(np.float32),
        "b_v": rng.uniform(-s, s, (EMBED,)).astype(np.float32),
        "W_o": rng.uniform(-s, s, (EMBED, EMBED)).astype(np.float32),
        "b_o": rng.uniform(-s, s, (EMBED,)).astype(np.float32),
    }
    out = kernel(**ins)
    print("kernel out", out.shape, out.dtype, float(np.abs(out).max()))



# revision 7
# speedup vs baseline: 1.2420x; 1.2420x over previous
"""Multi-head causal attention (B=2, T=2048, D=1024, H=16) on 8 Trainium2
NeuronCores.

Sharding: core c handles batch b = c//4 and head group g = c%4 (4 heads,
o-columns [256g, 256g+256)).  Host pre-transposes x and the weight slices so
every matmul operand arrives in contraction-major layout (all bf16); each core
computes its partial output projection y_part = att_part @ W_o.T[cols] and the
host sums the 4 partials per batch and adds b_o.

Per-core device program:
  Q^T,K^T  [o,t] = wT.T @ x^T    (Q scaled by 1/8, biases added on copy-out)
  V        [t,o] = x^T.T @ wvT   (+ ones-column per head for softmax denom)
  S^T      [k,q] = K^T_h.T @ Q^T_h   (two heads packed in PE row groups)
  P = exp(S^T)   (causal: lower-tri blocks only, diag blocks masked)
  O^T[dv,q], D[q] = [V_h|1].T @ P    (denominator via the ones column)
  att^T = O^T * bcast(1/D)           (rank-1 PE broadcast matmul)
  y_part[t,:] = att^T.T @ woT

PSUM pools are per-phase (scores / PV-accum / projections / output) so the
greedy Tile scheduler can hoist next-span projection matmuls into the
ACT-bound attention windows instead of idling the PE.
"""
import sys

for _p in ("/opt/trn_rl_repo", "/root/.axon_site/_ro/trn_rl_repo"):
    if _p not in sys.path:
        sys.path.insert(0, _p)

import numpy as np

import concourse.bass as bass
import concourse.tile as tile
from concourse import bacc, mybir

F32 = mybir.dt.float32
BF16 = mybir.dt.bfloat16
NP_BF16 = mybir.dt.np(BF16)

N_CORES = 8
EMBED = 1024
NH_CORE = 4          # heads per core
DH = 64              # head dim
OC = NH_CORE * DH    # 256 o-columns per core
KC = EMBED // 128    # 8 contraction chunks
NO = OC // 128       # 2 o-tiles of 128


def build_body(tc, aps, T, skip=()):
    nc = tc.nc
    P = 128
    SPAN = min(512, T)
    NSPAN = T // SPAN
    TPS = SPAN // P      # k/q tiles per span
    NT = T // P

    xT, wqT, wkT, wvT, woT, bq, bk, bv, y = (
        aps["xT"], aps["wqT"], aps["wkT"], aps["wvT"], aps["woT"],
        aps["bq"], aps["bk"], aps["bv"], aps["y"],
    )

    sb = aps["sb_pool"]
    psA = aps["psA"]   # scores S^T tiles           (2 buf x [128,1024] = 4 banks)
    psO = aps["psO"]   # PV accumulators            (2 buf x [65,512]   = 2 banks)
    psW = aps["psW"]   # projection/bcast/y tiles   (2 buf x [128,512]  = 2 banks)

    Exp = mybir.ActivationFunctionType.Exp
    add, mult = mybir.AluOpType.add, mybir.AluOpType.mult

    # ---- constants ----
    ones_f = sb.tile([128, 128], F32, tag="onesf")
    nc.gpsimd.memset(ones_f[:], 1.0)
    ones_sb = sb.tile([128, 128], BF16, tag="ones")
    nc.vector.tensor_copy(ones_sb[:], ones_f[:])
    tri_sb = sb.tile([128, 128], BF16, tag="tri")
    nc.gpsimd.memset(tri_sb[:], 1.0)
    # keep (q - k) >= 0 (k on partitions, q on free), else 0.0
    nc.gpsimd.affine_select(
        out=tri_sb[:], in_=tri_sb[:], compare_op=mybir.AluOpType.is_ge,
        fill=0.0, base=0, pattern=[[1, 128]], channel_multiplier=-1,
    )

    # ---- input loads (weights + per-span x chunks, all bf16) ----
    bq_sb = sb.tile([128, NO], F32, tag="bq")
    nc.sync.dma_start(bq_sb[:], bq.rearrange("(mo p) -> p mo", p=P))
    bk_sb = sb.tile([128, NO], F32, tag="bk")
    nc.sync.dma_start(bk_sb[:], bk.rearrange("(mo p) -> p mo", p=P))
    bv_sb = sb.tile([1, OC], BF16, tag="bv")
    nc.sync.dma_start(bv_sb[:], bv.rearrange("(a o) -> a o", a=1))
    wv_sb = sb.tile([128, KC, OC], BF16, tag="wv")
    nc.sync.dma_start(wv_sb[:], wvT.rearrange("(kc p) o -> p kc o", p=P))
    wq_sb = sb.tile([128, KC, OC], BF16, tag="wq")
    nc.sync.dma_start(wq_sb[:], wqT.rearrange("(kc p) o -> p kc o", p=P))
    wk_sb = sb.tile([128, KC, OC], BF16, tag="wk")
    nc.sync.dma_start(wk_sb[:], wkT.rearrange("(kc p) o -> p kc o", p=P))

    # x^T loaded per (span, kc) so span-0 compute starts after ~1/4 of x
    xv = xT.rearrange("(kc p) (s c) -> kc s p c", p=P, c=SPAN)
    xsp = []
    for s in range(NSPAN):
        row = []
        for kc in range(KC):
            t = sb.tile([128, SPAN], BF16, tag="xt", bufs=NSPAN * KC)
            nc.sync.dma_start(t[:], xv[kc, s])
            row.append(t)
        xsp.append(row)

    wo_sb = sb.tile([128, NO, EMBED], BF16, tag="wo")
    nc.sync.dma_start(wo_sb[:], woT.rearrange("(kc p) o -> p kc o", p=P))

    QT_sb = sb.tile([128, NO, T], BF16, tag="qt")
    KT_sb = sb.tile([128, NO, T], BF16, tag="kt")
    V_sb = sb.tile([128, NT, NH_CORE * (DH + 1)], BF16, tag="v")
    attT_sb = sb.tile([128, NO, T], BF16, tag="att")

    # softmax-denominator ones columns, written once per rep
    nc.gpsimd.memset(
        V_sb[:].rearrange("p t (h d) -> p t h d", d=DH + 1)[:, :, :, DH:DH + 1],
        1.0,
    )

    # ---- emission helpers (each emits one short PE chain + its copy-out) ----
    def v_chain(s, j):
        """V projection for t-tile j of span s (ones column via bias matmul)."""
        ti = TPS * s + j
        pv = psW.tile([128, OC], F32, tag="pw", bufs=2)
        for kc in range(KC):
            nc.tensor.matmul(
                pv[:], xsp[s][kc][:, j * 128:(j + 1) * 128], wv_sb[:, kc, :],
                start=(kc == 0), stop=False,
            )
        nc.tensor.matmul(
            pv[:], ones_sb[0:1, 0:128], bv_sb[0:1, :],
            start=False, stop=True,
        )
        nc.vector.tensor_copy(
            V_sb[:, ti, :].rearrange("p (h d) -> p h d", d=DH + 1)[:, :, 0:DH],
            pv[:].rearrange("p (h d) -> p h d", d=DH),
        )

    def qk_chain(s, which, mo):
        """Q (which=0) / K (which=1) projection chunk mo of span s."""
        dst, wsb, bias_sb = (
            (QT_sb, wq_sb, bq_sb), (KT_sb, wk_sb, bk_sb),
        )[which]
        pt = psW.tile([128, SPAN], F32, tag="pw", bufs=2)
        for kc in range(KC):
            nc.tensor.matmul(
                pt[:],
                wsb[:, kc, mo * 128:(mo + 1) * 128],
                xsp[s][kc][:, 0:SPAN],
                start=(kc == 0), stop=(kc == KC - 1),
            )
        if which == 0:
            nc.vector.tensor_scalar(
                dst[:, mo, s * SPAN:(s + 1) * SPAN], pt[:],
                bias_sb[:, mo:mo + 1], 0.125, add, mult,
            )
        else:
            nc.vector.tensor_scalar_add(
                dst[:, mo, s * SPAN:(s + 1) * SPAN], pt[:],
                bias_sb[:, mo:mo + 1],
            )

    def y_chain(s, ti, no):
        """Output projection chunk (t-tile ti, 512 embed cols no) of span s."""
        py = psW.tile([128, 512], F32, tag="pw", bufs=2)
        for kc2 in range(NO):
            nc.tensor.matmul(
                py[:],
                attT_sb[:, kc2, ti * 128:(ti + 1) * 128],
                wo_sb[:, kc2, no * 512:(no + 1) * 512],
                start=(kc2 == 0), stop=(kc2 == NO - 1),
            )
        ysb = sb.tile([128, 512], BF16, tag="y", bufs=3)
        nc.vector.tensor_copy(ysb[:], py[:])
        nc.sync.dma_start(
            y[ti * 128:(ti + 1) * 128, no * 512:(no + 1) * 512], ysb[:],
        )

    def proj_chains(s):
        """All projection chains for span s, in emission order."""
        out = []
        for j in range(TPS):
            out.append(lambda j=j: v_chain(s, j))
        for which in (0, 1):
            for mo in range(NO):
                out.append(lambda w=which, m=mo: qk_chain(s, w, m))
        return out

    def y_chains(s):
        if "yproj" in skip:
            return []
        return [lambda t=ti, n=no: y_chain(s, t, n)
                for ti in range(TPS * s, TPS * (s + 1))
                for no in range(EMBED // 512)]

    def attn_unit(s, hp, poh, idx, kt, nkt):
        """One attention unit: S^T pair matmul -> exp -> mask -> PV pair."""
        j = kt - TPS * s if kt >= TPS * s else None
        lo = 128 * j if j is not None else 0
        pstile = psA.tile([128, 2 * SPAN], F32, tag="ps", bufs=2)
        for hh in range(2 if "smm" not in skip else 0):
            h = 2 * hp + hh
            bp = 64 * (h % 2)
            nc.tensor.matmul(
                pstile[:, SPAN * hh + lo:SPAN * hh + SPAN],
                KT_sb[bp:bp + DH, h // 2, kt * 128:(kt + 1) * 128],
                QT_sb[bp:bp + DH, h // 2, SPAN * s + lo:SPAN * (s + 1)],
                start=True, stop=True,
            )
        pb = sb.tile([128, 2 * SPAN], BF16, tag="p", bufs=4)
        if "exp" not in skip:
            if lo == 0:
                nc.scalar.activation(pb[:], pstile[:], Exp)
            else:
                nc.scalar.activation(
                    pb[:].rearrange("q (h c) -> q h c", h=2)[:, :, lo:SPAN],
                    pstile[:].rearrange("q (h c) -> q h c", h=2)[:, :, lo:SPAN],
                    Exp,
                )
        if j is not None and "mask" not in skip:
            for hh in range(2):
                blk = pb[:, SPAN * hh + lo:SPAN * hh + lo + 128]
                nc.vector.tensor_mul(blk, blk, tri_sb[:])
        for hh in range(2 if "pv" not in skip else 0):
            h = 2 * hp + hh
            nc.tensor.matmul(
                poh[hh][0:DH + 1, lo:SPAN],
                V_sb[:, kt, (DH + 1) * h:(DH + 1) * (h + 1)],
                pb[:, SPAN * hh + lo:SPAN * hh + SPAN],
                start=(idx == 0), stop=(idx == nkt - 1),
            )

    def normalize(s, hp, poh):
        """att^T = O^T * bcast(1 / D) for both heads of pair hp."""
        rb = sb.tile([128, 2 * SPAN], BF16, tag="rb", bufs=2)
        bc = sb.tile([DH, 2 * SPAN], BF16, tag="bc", bufs=2)
        for hh in range(2):
            with nc.allow_low_precision(reason="bf16 recip for PE bcast"):
                nc.vector.reciprocal(
                    rb[DH:DH + 1, SPAN * hh:SPAN * (hh + 1)],
                    poh[hh][DH:DH + 1, 0:SPAN],
                )
            pbk = psW.tile([DH, 512], F32, tag="pw", bufs=2)
            nc.tensor.matmul(
                pbk[:, 0:SPAN],
                ones_sb[64:65, 0:DH],
                rb[64:65, SPAN * hh:SPAN * (hh + 1)],
                start=True, stop=True,
            )
            nc.scalar.copy(
                bc[:, SPAN * hh:SPAN * (hh + 1)], pbk[:, 0:SPAN],
            )
            h = 2 * hp + hh
            bp = 64 * (h % 2)
            nc.vector.tensor_mul(
                attT_sb[bp:bp + DH, h // 2, SPAN * s:SPAN * (s + 1)],
                poh[hh][0:DH, 0:SPAN],
                bc[:, SPAN * hh:SPAN * (hh + 1)],
            )

    # ---- prologue: projections for span 0 (nothing to hide them behind) ----
    for ch in proj_chains(0):
        ch()

    # ---- main loop: attention(s) with projections(s+1) and Y(s-1)
    #      interleaved between units so the scheduler can fill ACT-bound
    #      gaps with PE work at matching priority ----
    for s in range(NSPAN):
        fillers = []
        if s + 1 < NSPAN:
            fillers += proj_chains(s + 1)
        if s >= 1:
            fillers += y_chains(s - 1)
        n_hp = NH_CORE // 2 if "att" not in skip else 0
        kts = list(range(TPS * s, TPS * s + TPS)) + list(range(0, TPS * s))
        n_units = max(1, n_hp * len(kts))
        u = nf = 0
        for hp in range(n_hp):
            poh = [psO.tile([DH + 1, 512], F32, tag="po", bufs=2,
                            name=f"po_{s}_{hp}_{i}") for i in range(2)]
            for idx, kt in enumerate(kts):
                attn_unit(s, hp, poh, idx, kt, len(kts))
                u += 1
                want = u * len(fillers) // n_units
                while nf < want:
                    fillers[nf]()
                    nf += 1
            normalize(s, hp, poh)
        while nf < len(fillers):
            fillers[nf]()
            nf += 1

    # ---- epilogue: last span's output projection ----
    for ch in y_chains(NSPAN - 1):
        ch()


def build_nc(T=2048, reps=1, skip=()):
    nc = bacc.Bacc("TRN2", target_bir_lowering=False, debug=False,
                   enable_asserts=False, num_devices=N_CORES)
    aps = {
        "xT": nc.dram_tensor("xT", (EMBED, T), BF16, kind="ExternalInput").ap(),
        "wqT": nc.dram_tensor("wqT", (EMBED, OC), BF16, kind="ExternalInput").ap(),
        "wkT": nc.dram_tensor("wkT", (EMBED, OC), BF16, kind="ExternalInput").ap(),
        "wvT": nc.dram_tensor("wvT", (EMBED, OC), BF16, kind="ExternalInput").ap(),
        "woT": nc.dram_tensor("woT", (OC, EMBED), BF16, kind="ExternalInput").ap(),
        "bq": nc.dram_tensor("bq", (OC,), F32, kind="ExternalInput").ap(),
        "bk": nc.dram_tensor("bk", (OC,), F32, kind="ExternalInput").ap(),
        "bv": nc.dram_tensor("bv", (OC,), BF16, kind="ExternalInput").ap(),
        "y": nc.dram_tensor("y", (T, EMBED), BF16, kind="ExternalOutput").ap(),
    }
    with tile.TileContext(nc) as tc:
        with tc.tile_pool(name="sb", bufs=1) as sb, \
             tc.tile_pool(name="psA", bufs=1, space="PSUM") as psA, \
             tc.tile_pool(name="psO", bufs=1, space="PSUM") as psO, \
             tc.tile_pool(name="psW", bufs=1, space="PSUM") as psW:
            aps["sb_pool"] = sb
            aps["psA"] = psA
            aps["psO"] = psO
            aps["psW"] = psW
            if reps == 1:
                build_body(tc, aps, T, skip=skip)
            else:
                hints = (mybir.EngineType.PE, mybir.EngineType.Activation,
                         mybir.EngineType.DVE, mybir.EngineType.SP,
                         mybir.EngineType.Pool)
                with tc.For_i(0, reps, 1, hint_engines=hints):
                    build_body(tc, aps, T, skip=skip)
    nc.compile()
    return nc


def shard_inputs(x, W_q, b_q, W_k, b_k, W_v, b_v, W_o, b_o=None):
    """Full inputs -> list of 8 per-core input dicts (bf16 operands)."""
    in_maps = []
    for c in range(N_CORES):
        b, g = divmod(c, 4)
        sl = slice(OC * g, OC * (g + 1))
        in_maps.append({
            "xT": np.ascontiguousarray(x[b].T).astype(NP_BF16),
            "wqT": np.ascontiguousarray(W_q[sl, :].T).astype(NP_BF16),
            "wkT": np.ascontiguousarray(W_k[sl, :].T).astype(NP_BF16),
            "wvT": np.ascontiguousarray(W_v[sl, :].T).astype(NP_BF16),
            "woT": np.ascontiguousarray(W_o[:, sl].T).astype(NP_BF16),
            "bq": np.ascontiguousarray(b_q[sl], dtype=np.float32),
            "bk": np.ascontiguousarray(b_k[sl], dtype=np.float32),
            "bv": np.ascontiguousarray(b_v[sl]).astype(NP_BF16),
        })
    return in_maps


def _make_runner(nc, n_cores=N_CORES):
    """Compile-once, run-many SPMD runner (mirrors bass2jax.run_bass_via_pjrt)."""
    import jax
    from jax.sharding import Mesh, PartitionSpec
    from jax.experimental.shard_map import shard_map
    from concourse import bass2jax

    bass2jax.install_neuronx_cc_hook()
    partition_name = nc.partition_id_tensor.name if nc.partition_id_tensor else None
    in_names, out_names, out_avals, zero_outs = [], [], [], []
    for alloc in nc.m.functions[0].allocations:
        if not isinstance(alloc, mybir.MemoryLocationSet):
            continue
        name = alloc.memorylocations[0].name
        if alloc.kind == "ExternalInput":
            if name != partition_name:
                in_names.append(name)
        elif alloc.kind == "ExternalOutput":
            out_names.append(name)
            shape = tuple(alloc.tensor_shape)
            dtype = mybir.dt.np(alloc.dtype)
            out_avals.append(jax.core.ShapedArray(shape, dtype))
            zero_outs.append(np.zeros(shape, dtype))
    n_params = len(in_names)
    n_outs = len(out_avals)
    in_names_all = list(in_names) + list(out_names)
    if partition_name is not None:
        in_names_all.append(partition_name)
    donate = tuple(range(n_params, n_params + n_outs))

    def _body(*args):
        operands = list(args)
        if partition_name is not None:
            operands.append(bass2jax.partition_id_tensor())
        outs = bass2jax._bass_exec_p.bind(
            *operands,
            out_avals=tuple(out_avals),
            in_names=tuple(in_names_all),
            out_names=tuple(out_names),
            lowering_input_output_aliases=(),
            sim_require_finite=True,
            sim_require_nnan=True,
            nc=nc,
        )
        return tuple(outs)

    devices = jax.devices()[:n_cores]
    assert len(devices) == n_cores
    mesh = Mesh(np.asarray(devices), ("core",))
    in_specs = (PartitionSpec("core"),) * (n_params + n_outs)
    out_specs = (PartitionSpec("core"),) * len(out_names)
    jitted = jax.jit(
        shard_map(_body, mesh=mesh, in_specs=in_specs, out_specs=out_specs,
                  check_rep=False),
        donate_argnums=donate, keep_unused=True,
    )

    from jax.sharding import NamedSharding

    class Runner:
        def __init__(self):
            self._in_dev = None
            self._out_dev = None

        def prepare(self, in_maps):
            per_core = [[np.asarray(m[name]) for name in in_names]
                        for m in in_maps]
            concat_in = [
                np.concatenate([per_core[c][i] for c in range(n_cores)], axis=0)
                for i in range(n_params)
            ]
            sh = NamedSharding(mesh, PartitionSpec("core"))
            self._in_dev = [jax.device_put(a, sh) for a in concat_in]
            concat_zeros = [np.concatenate([z] * n_cores, axis=0)
                            for z in zero_outs]
            self._out_dev = [jax.device_put(a, sh) for a in concat_zeros]
            for a in self._in_dev + self._out_dev:
                a.block_until_ready()

        def execute(self):
            outs = jitted(*self._in_dev, *self._out_dev)
            for a in outs:
                a.block_until_ready()
            self._out_dev = list(outs)

        def fetch(self):
            out_arrs = [np.asarray(a) for a in self._out_dev]
            results = []
            for c in range(n_cores):
                m = {}
                for i, name in enumerate(out_names):
                    per_len = out_arrs[i].shape[0] // n_cores
                    m[name] = out_arrs[i][c * per_len:(c + 1) * per_len]
                results.append(m)
            return results

        def run(self, in_maps):
            self.prepare(in_maps)
            self.execute()
            return self.fetch()

    return Runner()


_CACHE = {}


def _get_runner(T=2048, reps=1):
    key = (T, reps)
    if key not in _CACHE:
        nc = build_nc(T=T, reps=reps)
        _CACHE[key] = _make_runner(nc)
    return _CACHE[key]


def kernel(**inputs):
    inputs = {k: np.asarray(v, dtype=np.float32) for k, v in inputs.items()}
    x = inputs["x"]
    B, T, C = x.shape
    in_maps = shard_inputs(**inputs)
    runner = _get_runner(T=T, reps=1)
    results = runner.run(in_maps)
    out = np.zeros((B, T, C), dtype=np.float32)
    for c in range(N_CORES):
        out[c // 4] += results[c]["y"].astype(np.float32)
    out += inputs["b_o"]
    return out


if __name__ == "__main__":
    rng = np.random.default_rng(0)
    s = 1.0 / np.sqrt(EMBED)
    ins = {
        "x": rng.standard_normal((2, 2048, EMBED), dtype=np.float32),
        "W_q": rng.uniform(-s, s, (EMBED, EMBED)).astype(np.float32),
        "b_q": rng.uniform(-s, s, (EMBED,)).astype(np.float32),
        "W_k": rng.uniform(-s, s, (EMBED, EMBED)).astype(np.float32),
        "b_k": rng.uniform(-s, s, (EMBED,)).astype(np.float32),
        "W_v": rng.uniform(-s, s, (EMBED, EMBED)).astype(np.float32),
        "b_v": rng.uniform(-s, s, (EMBED,)).astype(np.float32),
        "W_o": rng.uniform(-s, s, (EMBED, EMBED)).astype(np.float32),
        "b_o": rng.uniform(-s, s, (EMBED,)).astype(np.float32),
    }
    out = kernel(**ins)
    print("kernel out", out.shape, out.dtype, float(np.abs(out).max()))


# revision 21
# speedup vs baseline: 1.4189x; 1.1424x over previous
"""Multi-head causal attention (B=2, T=2048, D=1024, H=16) on 8 Trainium2
NeuronCores.

Sharding: core c handles batch b = c//4 and head group g = c%4 (4 heads,
o-columns [256g, 256g+256)).  Host pre-transposes x and the weight slices so
every matmul operand arrives in contraction-major layout (all bf16); each core
computes its partial output projection y_part = att_part @ W_o.T[cols] and the
host sums the 4 partials per batch and adds b_o.

Per-core device program:
  Q^T,K^T  [o,t] = wT.T @ x^T    (Q scaled by 1/8, biases added on copy-out)
  V        [t,o] = x^T.T @ wvT   (+ ones-column per head for softmax denom)
  S^T      [k,q] = K^T_h.T @ Q^T_h   (two heads packed in PE row groups)
  P = exp(S^T)   (causal: lower-tri blocks only, diag blocks masked)
  O^T[dv,q], D[q] = [V_h|1].T @ P    (denominator via the ones column)
  att^T = O^T * bcast(1/D)           (rank-1 PE broadcast matmul)
  y_part[t,:] = att^T.T @ woT

PSUM pools are per-phase (scores / PV-accum / projections / output) so the
greedy Tile scheduler can hoist next-span projection matmuls into the
ACT-bound attention windows instead of idling the PE.
"""
import sys

for _p in ("/opt/trn_rl_repo", "/root/.axon_site/_ro/trn_rl_repo"):
    if _p not in sys.path:
        sys.path.insert(0, _p)

import numpy as np

import concourse.bass as bass
import concourse.tile as tile
from concourse import bacc, mybir

F32 = mybir.dt.float32
BF16 = mybir.dt.bfloat16
NP_BF16 = mybir.dt.np(BF16)

N_CORES = 8
EMBED = 1024
NH_CORE = 4          # heads per core
DH = 64              # head dim
OC = NH_CORE * DH    # 256 o-columns per core
KC = EMBED // 128    # 8 contraction chunks
NO = OC // 128       # 2 o-tiles of 128


def build_body(tc, aps, T, skip=()):
    nc = tc.nc
    P = 128
    SPAN = min(512, T)
    NSPAN = T // SPAN
    TPS = SPAN // P      # k/q tiles per span
    NT = T // P

    xT, wqT, wkT, wvT, woT, bq, bk, y = (
        aps["xT"], aps["wqT"], aps["wkT"], aps["wvT"], aps["woT"],
        aps["bq"], aps["bk"], aps["y"],
    )

    sb = aps["sb_pool"]
    psA = aps["psA"]   # scores S^T tiles           (2 buf x [128,1024] = 4 banks)
    psO = aps["psO"]   # PV accumulators            (2 buf x [65,512]   = 2 banks)
    psW = aps["psW"]   # projection/bcast/y tiles   (2 buf x [128,512]  = 2 banks)

    Exp = mybir.ActivationFunctionType.Exp
    add, mult = mybir.AluOpType.add, mybir.AluOpType.mult

    # ---- constants ----
    ones_f = sb.tile([128, 128], F32, tag="onesf")
    nc.gpsimd.memset(ones_f[:], 1.0)
    ones_sb = sb.tile([128, 128], BF16, tag="ones")
    nc.vector.tensor_copy(ones_sb[:], ones_f[:])
    tri_sb = sb.tile([128, 128], BF16, tag="tri")
    nc.gpsimd.memset(tri_sb[:], 1.0)
    # keep (q - k) >= 0 (k on partitions, q on free), else 0.0
    nc.gpsimd.affine_select(
        out=tri_sb[:], in_=tri_sb[:], compare_op=mybir.AluOpType.is_ge,
        fill=0.0, base=0, pattern=[[1, 128]], channel_multiplier=-1,
    )

    # ---- input loads (weights + per-span x chunks, all bf16) ----
    bq_sb = sb.tile([128, NO], F32, tag="bq")
    nc.sync.dma_start(bq_sb[:], bq.rearrange("(mo p) -> p mo", p=P))
    bk_sb = sb.tile([128, NO], F32, tag="bk")
    nc.sync.dma_start(bk_sb[:], bk.rearrange("(mo p) -> p mo", p=P))
    wv_sb = sb.tile([128, KC, OC], BF16, tag="wv")
    nc.sync.dma_start(wv_sb[:], wvT.rearrange("(kc p) o -> p kc o", p=P))
    wq_sb = sb.tile([128, KC, OC], BF16, tag="wq")
    nc.sync.dma_start(wq_sb[:], wqT.rearrange("(kc p) o -> p kc o", p=P))
    wk_sb = sb.tile([128, KC, OC], BF16, tag="wk")
    nc.sync.dma_start(wk_sb[:], wkT.rearrange("(kc p) o -> p kc o", p=P))

    # x^T loaded per (span, kc) so span-0 compute starts after ~1/4 of x
    xv = xT.rearrange("(kc p) (s c) -> kc s p c", p=P, c=SPAN)
    xsp = []
    for s in range(NSPAN):
        row = []
        for kc in range(KC):
            t = sb.tile([128, SPAN], BF16, tag="xt", bufs=NSPAN * KC)
            nc.sync.dma_start(t[:], xv[kc, s])
            row.append(t)
        xsp.append(row)

    wo_sb = sb.tile([128, NO, EMBED], BF16, tag="wo")
    nc.sync.dma_start(wo_sb[:], woT.rearrange("(kc p) o -> p kc o", p=P))

    QT_sb = sb.tile([128, NO, T], BF16, tag="qt")
    KT_sb = sb.tile([128, NO, T], BF16, tag="kt")
    V_sb = sb.tile([128, NT, NH_CORE * (DH + 1)], BF16, tag="v")
    attT_sb = sb.tile([128, NO, T], BF16, tag="att")

    # softmax-denominator ones columns, written once per rep
    nc.gpsimd.memset(
        V_sb[:].rearrange("p t (h d) -> p t h d", d=DH + 1)[:, :, :, DH:DH + 1],
        1.0,
    )

    # ---- emission helpers (each emits one short PE chain + its copy-out) ----
    def v_chain(s, j):
        """V projection for t-tile j of span s (ones column via bias matmul)."""
        if "proj" in skip:
            return
        ti = TPS * s + j
        pv = psW.tile([128, OC], F32, tag="pw", bufs=2)
        for kc in range(KC):
            nc.tensor.matmul(
                pv[:], xsp[s][kc][:, j * 128:(j + 1) * 128], wv_sb[:, kc, :],
                start=(kc == 0), stop=(kc == KC - 1),
            )
        nc.vector.tensor_copy(
            V_sb[:, ti, :].rearrange("p (h d) -> p h d", d=DH + 1)[:, :, 0:DH],
            pv[:].rearrange("p (h d) -> p h d", d=DH),
        )

    def qk_chain(s, which, mo):
        """Q (which=0) / K (which=1) projection chunk mo of span s."""
        if "proj" in skip:
            return
        dst, wsb, bias_sb = (
            (QT_sb, wq_sb, bq_sb), (KT_sb, wk_sb, bk_sb),
        )[which]
        pt = psW.tile([128, SPAN], F32, tag="pw", bufs=2)
        for kc in range(KC):
            nc.tensor.matmul(
                pt[:],
                wsb[:, kc, mo * 128:(mo + 1) * 128],
                xsp[s][kc][:, 0:SPAN],
                start=(kc == 0), stop=(kc == KC - 1),
            )
        if which == 0:
            nc.vector.tensor_scalar(
                dst[:, mo, s * SPAN:(s + 1) * SPAN], pt[:],
                bias_sb[:, mo:mo + 1], 0.125, add, mult,
            )
        else:
            nc.vector.tensor_scalar_add(
                dst[:, mo, s * SPAN:(s + 1) * SPAN], pt[:],
                bias_sb[:, mo:mo + 1],
            )

    def y_chain(s, ti, no):
        """Output projection chunk (t-tile ti, 512 embed cols no) of span s."""
        py = psW.tile([128, 512], F32, tag="pw", bufs=2)
        for kc2 in range(NO):
            nc.tensor.matmul(
                py[:],
                attT_sb[:, kc2, ti * 128:(ti + 1) * 128],
                wo_sb[:, kc2, no * 512:(no + 1) * 512],
                start=(kc2 == 0), stop=(kc2 == NO - 1),
            )
        ysb = sb.tile([128, 512], BF16, tag="y", bufs=3)
        nc.vector.tensor_copy(ysb[:], py[:])
        nc.sync.dma_start(
            y[ti * 128:(ti + 1) * 128, no * 512:(no + 1) * 512], ysb[:],
        )

    def proj_chains(s):
        """All projection chains for span s, in emission order."""
        out = []
        for j in range(TPS):
            out.append(lambda j=j: v_chain(s, j))
        for which in (0, 1):
            for mo in range(NO):
                out.append(lambda w=which, m=mo: qk_chain(s, w, m))
        return out

    def y_chains(s):
        if "yproj" in skip:
            return []
        return [lambda t=ti, n=no: y_chain(s, t, n)
                for ti in range(TPS * s, TPS * (s + 1))
                for no in range(EMBED // 512)]

    def attn_unit(s, hp, poh, idx, kt, nkt):
        """One attention unit: S^T pair matmul -> exp -> mask -> PV pair."""
        j = kt - TPS * s if kt >= TPS * s else None
        lo = 128 * j if j is not None else 0
        pstile = psA.tile([128, 2 * SPAN], F32, tag="ps", bufs=2)
        for hh in range(2 if "smm" not in skip else 0):
            h = 2 * hp + hh
            bp = 64 * (h % 2)
            nc.tensor.matmul(
                pstile[:, SPAN * hh + lo:SPAN * hh + SPAN],
                KT_sb[bp:bp + DH, h // 2, kt * 128:(kt + 1) * 128],
                QT_sb[bp:bp + DH, h // 2, SPAN * s + lo:SPAN * (s + 1)],
                start=True, stop=True,
            )
        pb = sb.tile([128, 2 * SPAN], BF16, tag="p", bufs=6)
        if "exp" not in skip:
            if lo == 0:
                nc.scalar.activation(pb[:], pstile[:], Exp)
            else:
                nc.scalar.activation(
                    pb[:].rearrange("q (h c) -> q h c", h=2)[:, :, lo:SPAN],
                    pstile[:].rearrange("q (h c) -> q h c", h=2)[:, :, lo:SPAN],
                    Exp,
                )
        if j is not None and "mask" not in skip:
            for hh in range(2):
                blk = pb[:, SPAN * hh + lo:SPAN * hh + lo + 128]
                nc.gpsimd.tensor_mul(blk, blk, tri_sb[:])
        for hh in range(2 if "pv" not in skip else 0):
            h = 2 * hp + hh
            nc.tensor.matmul(
                poh[hh][0:DH + 1, lo:SPAN],
                V_sb[:, kt, (DH + 1) * h:(DH + 1) * (h + 1)],
                pb[:, SPAN * hh + lo:SPAN * hh + SPAN],
                start=(idx == 0), stop=(idx == nkt - 1),
            )

    def normalize(s, hp, poh):
        """att^T = O^T * bcast(1 / D) for both heads of pair hp."""
        rb = sb.tile([128, 2 * SPAN], BF16, tag="rb", bufs=2)
        bc = sb.tile([DH, 2 * SPAN], BF16, tag="bc", bufs=2)
        for hh in range(2):
            with nc.allow_low_precision(reason="bf16 recip for PE bcast"):
                nc.vector.reciprocal(
                    rb[DH:DH + 1, SPAN * hh:SPAN * (hh + 1)],
                    poh[hh][DH:DH + 1, 0:SPAN],
                )
            pbk = psW.tile([DH, 512], F32, tag="pw", bufs=2)
            nc.tensor.matmul(
                pbk[:, 0:SPAN],
                ones_sb[64:65, 0:DH],
                rb[64:65, SPAN * hh:SPAN * (hh + 1)],
                start=True, stop=True,
            )
            nc.scalar.copy(
                bc[:, SPAN * hh:SPAN * (hh + 1)], pbk[:, 0:SPAN],
            )
            h = 2 * hp + hh
            bp = 64 * (h % 2)
            nc.vector.tensor_mul(
                attT_sb[bp:bp + DH, h // 2, SPAN * s:SPAN * (s + 1)],
                poh[hh][0:DH, 0:SPAN],
                bc[:, SPAN * hh:SPAN * (hh + 1)],
            )

    # ---- prologue: projections for span 0 (nothing to hide them behind) ----
    for ch in proj_chains(0):
        ch()

    # ---- main loop: attention(s) with projections(s+1) and Y(s-1)
    #      interleaved between units so the scheduler can fill ACT-bound
    #      gaps with PE work at matching priority ----
    for s in range(NSPAN):
        fillers = []
        if s + 1 < NSPAN:
            fillers += proj_chains(s + 1)
        if s >= 1:
            fillers += y_chains(s - 1)
        n_hp = NH_CORE // 2 if "att" not in skip else 0
        kts = list(range(TPS * s, TPS * s + TPS)) + list(range(0, TPS * s))
        n_units = max(1, n_hp * len(kts))
        u = nf = 0
        for hp in range(n_hp):
            poh = [psO.tile([DH + 1, 512], F32, tag="po", bufs=2,
                            name=f"po_{s}_{hp}_{i}") for i in range(2)]
            for idx, kt in enumerate(kts):
                attn_unit(s, hp, poh, idx, kt, len(kts))
                u += 1
                want = u * len(fillers) // n_units
                while nf < want:
                    fillers[nf]()
                    nf += 1
            normalize(s, hp, poh)
        while nf < len(fillers):
            fillers[nf]()
            nf += 1

    # ---- epilogue: last span's output projection ----
    for ch in y_chains(NSPAN - 1):
        ch()


def build_nc(T=2048, reps=1, skip=(), unroll=1):
    nc = bacc.Bacc("TRN2", target_bir_lowering=False, debug=False,
                   enable_asserts=False, num_devices=N_CORES)
    aps = {
        "xT": nc.dram_tensor("xT", (EMBED, T), BF16, kind="ExternalInput").ap(),
        "wqT": nc.dram_tensor("wqT", (EMBED, OC), BF16, kind="ExternalInput").ap(),
        "wkT": nc.dram_tensor("wkT", (EMBED, OC), BF16, kind="ExternalInput").ap(),
        "wvT": nc.dram_tensor("wvT", (EMBED, OC), BF16, kind="ExternalInput").ap(),
        "woT": nc.dram_tensor("woT", (OC, EMBED), BF16, kind="ExternalInput").ap(),
        "bq": nc.dram_tensor("bq", (OC,), F32, kind="ExternalInput").ap(),
        "bk": nc.dram_tensor("bk", (OC,), F32, kind="ExternalInput").ap(),
        "y": nc.dram_tensor("y", (T, EMBED), BF16, kind="ExternalOutput").ap(),
    }
    with tile.TileContext(nc) as tc:
        with tc.tile_pool(name="sb", bufs=1) as sb, \
             tc.tile_pool(name="psA", bufs=1, space="PSUM") as psA, \
             tc.tile_pool(name="psO", bufs=1, space="PSUM") as psO, \
             tc.tile_pool(name="psW", bufs=1, space="PSUM") as psW:
            aps["sb_pool"] = sb
            aps["psA"] = psA
            aps["psO"] = psO
            aps["psW"] = psW
            if reps == 1:
                for _ in range(unroll):
                    build_body(tc, aps, T, skip=skip)
            else:
                hints = (mybir.EngineType.PE, mybir.EngineType.Activation,
                         mybir.EngineType.DVE, mybir.EngineType.SP,
                         mybir.EngineType.Pool)
                with tc.For_i(0, reps, 1, hint_engines=hints):
                    for _ in range(unroll):
                        build_body(tc, aps, T, skip=skip)
    nc.compile()
    return nc


def shard_inputs(x, W_q, b_q, W_k, b_k, W_v, b_v, W_o, b_o=None):
    """Full inputs -> list of 8 per-core input dicts (bf16 operands)."""
    in_maps = []
    for c in range(N_CORES):
        b, g = divmod(c, 4)
        sl = slice(OC * g, OC * (g + 1))
        in_maps.append({
            "xT": np.ascontiguousarray(x[b].T).astype(NP_BF16),
            "wqT": np.ascontiguousarray(W_q[sl, :].T).astype(NP_BF16),
            "wkT": np.ascontiguousarray(W_k[sl, :].T).astype(NP_BF16),
            "wvT": np.ascontiguousarray(W_v[sl, :].T).astype(NP_BF16),
            "woT": np.ascontiguousarray(W_o[:, sl].T).astype(NP_BF16),
            "bq": np.ascontiguousarray(b_q[sl], dtype=np.float32),
            "bk": np.ascontiguousarray(b_k[sl], dtype=np.float32),
        })
    return in_maps


def _make_runner(nc, n_cores=N_CORES):
    """Compile-once, run-many SPMD runner (mirrors bass2jax.run_bass_via_pjrt)."""
    import jax
    from jax.sharding import Mesh, PartitionSpec
    from jax.experimental.shard_map import shard_map
    from concourse import bass2jax

    bass2jax.install_neuronx_cc_hook()
    partition_name = nc.partition_id_tensor.name if nc.partition_id_tensor else None
    in_names, out_names, out_avals, zero_outs = [], [], [], []
    for alloc in nc.m.functions[0].allocations:
        if not isinstance(alloc, mybir.MemoryLocationSet):
            continue
        name = alloc.memorylocations[0].name
        if alloc.kind == "ExternalInput":
            if name != partition_name:
                in_names.append(name)
        elif alloc.kind == "ExternalOutput":
            out_names.append(name)
            shape = tuple(alloc.tensor_shape)
            dtype = mybir.dt.np(alloc.dtype)
            out_avals.append(jax.core.ShapedArray(shape, dtype))
            zero_outs.append(np.zeros(shape, dtype))
    n_params = len(in_names)
    n_outs = len(out_avals)
    in_names_all = list(in_names) + list(out_names)
    if partition_name is not None:
        in_names_all.append(partition_name)
    donate = tuple(range(n_params, n_params + n_outs))

    def _body(*args):
        operands = list(args)
        if partition_name is not None:
            operands.append(bass2jax.partition_id_tensor())
        outs = bass2jax._bass_exec_p.bind(
            *operands,
            out_avals=tuple(out_avals),
            in_names=tuple(in_names_all),
            out_names=tuple(out_names),
            lowering_input_output_aliases=(),
            sim_require_finite=True,
            sim_require_nnan=True,
            nc=nc,
        )
        return tuple(outs)

    devices = jax.devices()[:n_cores]
    assert len(devices) == n_cores
    mesh = Mesh(np.asarray(devices), ("core",))
    in_specs = (PartitionSpec("core"),) * (n_params + n_outs)
    out_specs = (PartitionSpec("core"),) * len(out_names)
    jitted = jax.jit(
        shard_map(_body, mesh=mesh, in_specs=in_specs, out_specs=out_specs,
                  check_rep=False),
        donate_argnums=donate, keep_unused=True,
    )

    from jax.sharding import NamedSharding

    class Runner:
        def __init__(self):
            self._in_dev = None
            self._out_dev = None

        def prepare(self, in_maps):
            per_core = [[np.asarray(m[name]) for name in in_names]
                        for m in in_maps]
            concat_in = [
                np.concatenate([per_core[c][i] for c in range(n_cores)], axis=0)
                for i in range(n_params)
            ]
            sh = NamedSharding(mesh, PartitionSpec("core"))
            self._in_dev = [jax.device_put(a, sh) for a in concat_in]
            concat_zeros = [np.concatenate([z] * n_cores, axis=0)
                            for z in zero_outs]
            self._out_dev = [jax.device_put(a, sh) for a in concat_zeros]
            for a in self._in_dev + self._out_dev:
                a.block_until_ready()

        def execute(self):
            outs = jitted(*self._in_dev, *self._out_dev)
            for a in outs:
                a.block_until_ready()
            self._out_dev = list(outs)

        def fetch(self):
            out_arrs = [np.asarray(a) for a in self._out_dev]
            results = []
            for c in range(n_cores):
                m = {}
                for i, name in enumerate(out_names):
                    per_len = out_arrs[i].shape[0] // n_cores
                    m[name] = out_arrs[i][c * per_len:(c + 1) * per_len]
                results.append(m)
            return results

        def run(self, in_maps):
            self.prepare(in_maps)
            self.execute()
            return self.fetch()

    return Runner()


_CACHE = {}


def _get_runner(T=2048, reps=1):
    key = (T, reps)
    if key not in _CACHE:
        nc = build_nc(T=T, reps=reps)
        _CACHE[key] = _make_runner(nc)
    return _CACHE[key]


def kernel(**inputs):
    inputs = {k: np.asarray(v, dtype=np.float32) for k, v in inputs.items()}
    x = inputs["x"]
    B, T, C = x.shape
    in_maps = shard_inputs(**inputs)
    runner = _get_runner(T=T, reps=1)
    results = runner.run(in_maps)
    out = np.zeros((B, T, C), dtype=np.float32)
    for c in range(N_CORES):
        out[c // 4] += results[c]["y"].astype(np.float32)
    # b_v passes through attention as a constant per-channel offset (softmax
    # rows sum to 1), so its contribution b_v @ W_o.T folds into the bias.
    out += inputs["b_o"] + inputs["b_v"] @ inputs["W_o"].T
    return out


if __name__ == "__main__":
    rng = np.random.default_rng(0)
    s = 1.0 / np.sqrt(EMBED)
    ins = {
        "x": rng.standard_normal((2, 2048, EMBED), dtype=np.float32),
        "W_q": rng.uniform(-s, s, (EMBED, EMBED)).astype(np.float32),
        "b_q": rng.uniform(-s, s, (EMBED,)).astype(np.float32),
        "W_k": rng.uniform(-s, s, (EMBED, EMBED)).astype(np.float32),
        "b_k": rng.uniform(-s, s, (EMBED,)).astype(np.float32),
        "W_v": rng.uniform(-s, s, (EMBED, EMBED)).astype(np.float32),
        "b_v": rng.uniform(-s, s, (EMBED,)).astype(np.float32),
        "W_o": rng.uniform(-s, s, (EMBED, EMBED)).astype(np.float32),
        "b_o": rng.uniform(-s, s, (EMBED,)).astype(np.float32),
    }
    out = kernel(**ins)
    print("kernel out", out.shape, out.dtype, float(np.abs(out).max()))
